# revision 13
# baseline (speedup 1.0000x reference)
"""GatedAttention Trainium2 kernel, 8-way tensor-parallel over heads.

Reference computation (B=1, S=2048, D=2048, H=16 heads, Hd=128):
  q,k,v = x @ {q,k,v}_w.T  (per-head split)
  scores = (q @ k.T) / sqrt(Hd), causal mask, softmax
  av = attn @ v
  gate = sigmoid(q @ gate_w.T + gate_b)       (per-head)
  y = concat_heads(av * gate) @ o_w.T

Sharding: 2 heads per core (column-parallel QKV/gate). The gated per-head
outputs are AllGathered in bf16 [feature, seq] layout, one AllGather per
512-wide q-chunk (both heads stacked: 256 rows x 512 cols, 256KB/rank).
o_proj is column-parallel over the gathered features; the host concatenates
the 8 output column slices.

Schedule notes:
 - Fully chunk-pipelined along the sequence: x^T is loaded one 512-column
   chunk at a time, and each chunk runs Q/K proj -> gates -> V proj ->
   attention -> store -> AllGather before the next chunk's attention. The
   first collective therefore triggers ~60us into the kernel and the four
   serialized collectives (~26us each on this LNC1 8-rank config) hide
   almost entirely under the remaining compute.
 - Softmax runs without max-subtraction (scores are small by construction).
   exp row-sums are accumulated per-block on the Vector engine (running
   tensor_add over the [j,q]-layout exp tiles) and reduced across the j
   partition axis with one gpsimd partition_all_reduce per block, keeping
   the PE free of M=1 ones-matmuls.
 - attds stores ride the gpsimd queue (the AllGather trigger that depends
   on them is right behind); the only DMAs on the sync queue after the
   input loads are the gathered-output reads, so a read blocked on its
   AllGather can never head-of-line-block an attds store.
"""

import numpy as np
import ml_dtypes

import concourse.bass as bass
import concourse.bass_isa as bass_isa
import concourse.mybir as mybir
import concourse.tile as tile
from concourse import bacc
from concourse.bass_utils import run_bass_kernel_spmd

BF16 = ml_dtypes.bfloat16
F32 = mybir.dt.float32
BF = mybir.dt.bfloat16
AF = mybir.ActivationFunctionType

N_CORES = 8
S = 2048          # sequence length
D = 2048          # model dim
H = 16            # total heads
HD = 128          # head dim
HPC = H // N_CORES                   # heads per core: 2
E = HPC * HD                         # 256 output dims per core
DC = D // 128                        # 16 contraction chunks
QCW = 512                            # q-chunk width
NQC = S // QCW                       # 4 q-chunks
SCALE = 1.0 / float(np.sqrt(HD))

_CACHED = {}


def _build(collective=True):
    nc = bacc.Bacc("TRN2", target_bir_lowering=False, debug=False,
                   num_devices=N_CORES if collective else 1,
                   enable_asserts=False)

    xt = nc.dram_tensor("xt", [D, S], BF, kind="ExternalInput")        # x^T
    wqt = nc.dram_tensor("wqt", [D, E], BF, kind="ExternalInput")      # q_w shard^T
    wkt = nc.dram_tensor("wkt", [D, E], BF, kind="ExternalInput")
    wvt = nc.dram_tensor("wvt", [D, E], BF, kind="ExternalInput")
    owt = nc.dram_tensor("owt", [D, E], BF, kind="ExternalInput")      # o_w shard^T
    gwt = nc.dram_tensor("gwt", [HD, HD], BF, kind="ExternalInput")    # gate_w^T
    gb = nc.dram_tensor("gb", [HD, 1], F32, kind="ExternalInput")      # gate bias
    trim = nc.dram_tensor("trim", [128, 128], BF, kind="ExternalInput")
    yt = nc.dram_tensor("yt", [E, S], F32, kind="ExternalOutput")      # y^T slice

    shared = "Shared" if collective else "Local"

    with tile.TileContext(nc) as tc:
        with tc.tile_pool(name="const", bufs=1) as const, \
             tc.tile_pool(name="work", bufs=2) as work, \
             tc.tile_pool(name="psum", bufs=1, space="PSUM") as psum, \
             tc.tile_pool(name="dram", bufs=1, space="DRAM") as dram:

            def pp(name):
                return psum.tile([128, QCW], F32, tag="pp", bufs=6, name=name)

            def pp_av(name):
                # attention-output accumulators get their own two banks:
                # they drain late (deferred epilogue reads them one block
                # later), and sharing the rotation would stall projection
                # chains on the epilogue's att-multiply
                return psum.tile([128, QCW], F32, tag="avp", bufs=2,
                                 name=name)

            # ---- weight loads ----
            wqts = const.tile([128, DC, E], BF, tag="wqts", name="wqts")
            wkts = const.tile([128, DC, E], BF, tag="wkts", name="wkts")
            wvts = const.tile([128, DC, E], BF, tag="wvts", name="wvts")
            owts = const.tile([128, DC, E], BF, tag="owts", name="owts")

            def _ldw(dst, src, half):
                sl = slice(half * 8, (half + 1) * 8)
                nc.sync.dma_start(
                    dst[:, sl, :],
                    src.ap()[half * 1024:(half + 1) * 1024, :]
                       .rearrange("(c p) e -> p c e", p=128))

            # x^T resident in SBUF, loaded in per-(q-chunk, dc) column tiles
            # (128KB each) so the transfers spread across all DMA queues in
            # q-chunk priority order and no projection chain ever catches
            # the stream mid-chunk
            xts = const.tile([128, DC, S], BF, tag="xts", name="xts")

            def emit_xload(qc):
                q0 = qc * QCW
                for d0 in range(DC):
                    nc.sync.dma_start(
                        xts[:, d0, q0:q0 + QCW],
                        xt.ap()[d0 * 128:(d0 + 1) * 128, q0:q0 + QCW])

            _ldw(wqts, wqt, 0)
            emit_xload(0)
            _ldw(wkts, wkt, 0)
            _ldw(wqts, wqt, 1)
            _ldw(wkts, wkt, 1)

            gwts = const.tile([HD, HD], BF, tag="gwts", name="gwts")
            gbs = const.tile([HD, 1], F32, tag="gbs", name="gbs")
            tris = const.tile([128, 128], BF, tag="tris", name="tris")
            nc.sync.dma_start(gwts[:], gwt.ap())
            nc.sync.dma_start(gbs[:], gb.ap())
            nc.sync.dma_start(tris[:], trim.ap())
            nc.sync.dma_start(wvts[:], wvt.ap().rearrange("(c p) e -> p c e", p=128))
            emit_xload(1)
            nc.sync.dma_start(owts[:], owt.ap().rearrange("(c p) e -> p c e", p=128))
            emit_xload(2)
            emit_xload(3)

            # ---- persistent activation tiles ----
            qts = const.tile([128, HPC, S], BF, tag="qts", name="qts")
            kts = const.tile([128, HPC, S], BF, tag="kts", name="kts")
            gts = const.tile([128, HPC, S], BF, tag="gts", name="gts")
            vts = const.tile([128, DC, E], BF, tag="vts", name="vts")

            attds = [dram.tile([E, QCW], BF, tag=f"attd{qc}", name=f"attd{qc}")
                     for qc in range(NQC)]
            outds = [dram.tile([N_CORES * E, QCW], BF, tag=f"outd{qc}",
                               addr_space=shared, name=f"outd{qc}")
                     for qc in range(NQC)]

            def emit_proj(wts, outts, h, qc):
                # one 16-matmul chain: outts[:, h, qc] = W_h^T x[:, qc]
                q0 = qc * QCW
                ppt = pp("qp")
                for dc in range(DC):
                    nc.tensor.matmul(
                        ppt[:], wts[:, dc, h * 128:(h + 1) * 128],
                        xts[:, dc, q0:q0 + QCW], start=(dc == 0),
                        stop=(dc == DC - 1))
                nc.vector.tensor_copy(out=outts[:, h, q0:q0 + QCW], in_=ppt[:])

            def emit_vchain(sc16):
                # V: [s(128), e] natural layout, one 16-matmul chain
                vp = pp("vp")
                for dc in range(DC):
                    nc.tensor.matmul(
                        vp[:, :E],
                        xts[:, dc, sc16 * 128:(sc16 + 1) * 128],
                        wvts[:, dc, :], start=(dc == 0), stop=(dc == DC - 1))
                nc.vector.tensor_copy(out=vts[:, sc16, :], in_=vp[:, :E])

            def emit_gate(h, qc):
                q0 = qc * QCW
                gp = pp("gp")
                nc.tensor.matmul(gp[:], gwts[:], qts[:, h, q0:q0 + QCW],
                                 start=True, stop=True)
                nc.scalar.activation(gts[:, h, q0:q0 + QCW], gp[:],
                                     AF.Sigmoid, bias=gbs[:, 0:1])

            def emit_ag(qc):
                if collective:
                    nc.gpsimd.collective_compute(
                        "AllGather", mybir.AluOpType.bypass,
                        replica_groups=[list(range(N_CORES))],
                        ins=[attds[qc][:].opt()], outs=[outds[qc][:].opt()])
                else:
                    nc.sync.dma_start(outds[qc][0:E, :], attds[qc][:])

            # Software-pipelined across blocks: each block's last AV matmuls
            # and its epilogue are emitted after the NEXT chunk of PE work
            # (projection chain or next block's first scores/exp), so the PE
            # never idles waiting for the tail exp on ACT.
            pend = None   # deferred tail of the previous block

            def emit_tail_av(t, k):
                # deferred AV for jj_l-1 (k=0) or jj_l (k=1, stop)
                (h, qc, avp, acc, exts_l, s0s, jj_l) = t
                jj = jj_l - 1 + k
                s0 = s0s[k]
                nc.tensor.matmul(
                    avp[:, s0:], vts[:, jj, h * 128:(h + 1) * 128],
                    exts_l[jj % 3][:, s0:], start=False, stop=(k == 1))

            def emit_tail(t):
                (h, qc, avp, acc, exts_l, s0s, jj_l) = t
                q0 = qc * QCW
                # softmax denominator: reduce the running exp-sum across the
                # j partition axis on the (otherwise idle) Pool engine
                sumb = work.tile([128, QCW], F32, tag="sumb", bufs=2,
                                 name="sumb")
                nc.gpsimd.partition_all_reduce(sumb[:], acc[:], channels=128,
                                               reduce_op=bass_isa.ReduceOp.add)
                rb = work.tile([128, QCW], F32, tag="rb", bufs=2, name="rb")
                nc.vector.reciprocal_approx_fast(out=rb[:], in_=sumb[:])
                gn = work.tile([128, QCW], BF, tag="gn", bufs=2, name="gn")
                nc.vector.tensor_mul(gn[:], gts[:, h, q0:q0 + QCW], rb[:])
                att = work.tile([128, QCW], BF, tag="att", bufs=2, name="att")
                nc.vector.tensor_mul(att[:], avp[:], gn[:])
                nc.gpsimd.dma_start(attds[qc][h * HD:(h + 1) * HD, :], att[:])
                if h == HPC - 1:
                    emit_ag(qc)

            for qc in range(NQC):
                # Q chains, with the previous q-chunk's deferred tail riding
                # between them
                emit_proj(wqts, qts, 0, qc)
                if pend is not None:
                    emit_tail_av(pend, 0)
                emit_proj(wqts, qts, 1, qc)
                if pend is not None:
                    emit_tail_av(pend, 1)
                    emit_tail(pend)
                    pend = None
                emit_proj(wkts, kts, 0, qc)
                emit_proj(wkts, kts, 1, qc)
                emit_gate(0, qc)
                emit_gate(1, qc)
                for sc16 in range(4 * qc, 4 * qc + 4):
                    emit_vchain(sc16)

                for h in range(HPC):
                    q0 = qc * QCW
                    scps = [pp("scp") for _ in range(3)]
                    avp = pp_av("avp")
                    njj = 4 * qc + 4
                    exts = [work.tile([128, QCW], BF, tag="ext", bufs=6,
                                      name="ext") for _ in range(3)]
                    acc = work.tile([128, QCW], F32, tag="acc", bufs=2,
                                    name="acc")

                    def s0_of(jj):
                        return max(0, (jj - 4 * qc) * 128)

                    def emit_av(jj):
                        s0 = s0_of(jj)
                        nc.tensor.matmul(
                            avp[:, s0:], vts[:, jj, h * 128:(h + 1) * 128],
                            exts[jj % 3][:, s0:],
                            start=(jj == 0), stop=False)

                    # scores run two jj ahead of AV so the PE never waits on
                    # the exp->mask chain; the exp-sum accumulates on Vector
                    for jj in range(njj):
                        off = jj - 4 * qc
                        s0 = s0_of(jj)
                        scp = scps[jj % 3]
                        ext = exts[jj % 3]
                        nc.tensor.matmul(
                            scp[:, s0:], kts[:, h, jj * 128:(jj + 1) * 128],
                            qts[:, h, q0 + s0:q0 + QCW], start=True, stop=True)
                        nc.scalar.activation(ext[:, s0:], scp[:, s0:],
                                             AF.Exp, scale=SCALE)
                        if off >= 0:
                            nc.vector.tensor_mul(ext[:, s0:s0 + 128],
                                                 ext[:, s0:s0 + 128], tris[:])
                        if jj == 0:
                            nc.vector.tensor_copy(out=acc[:], in_=ext[:])
                        else:
                            nc.vector.tensor_add(acc[:, s0:], acc[:, s0:],
                                                 ext[:, s0:])
                        if pend is not None:
                            if jj == 0:
                                emit_tail_av(pend, 0)
                            elif jj == 1:
                                emit_tail_av(pend, 1)
                                emit_tail(pend)
                                pend = None
                        if jj >= 2:
                            emit_av(jj - 2)
                    pend = (h, qc, avp, acc, exts,
                            (s0_of(njj - 2), s0_of(njj - 1)), njj - 1)
            # flush the final block so the last store - and with it the
            # last AllGather - issues immediately
            emit_tail_av(pend, 0)
            emit_tail_av(pend, 1)
            emit_tail(pend)
            pend = None

            # ---- o_proj: y^T[e', s] = sum_f o_w[cs+e', f] out^T[f, s] ----
            # per q-chunk as the gathers land; the gathered reads are the
            # only DMAs left on the sync queue so their AllGather waits
            # cannot block anything else
            for qc in range(NQC):
                outsb = work.tile([128, 2 * N_CORES, QCW], BF, tag="outsb",
                                  bufs=2, name="outsb")
                # one DMA per 128-row feature chunk so the 2MB gathered read
                # spreads across all DMA queues instead of serializing on one
                for g in range(2 * N_CORES):
                    nc.sync.dma_start(
                        outsb[:, g, :],
                        outds[qc][g * 128:(g + 1) * 128, :])
                for ec in range(HPC):
                    yp = pp("yp")
                    for g in range(2 * N_CORES):
                        nc.tensor.matmul(
                            yp[:],
                            owts[:, g, ec * 128:(ec + 1) * 128],
                            outsb[:, g, :],
                            start=(g == 0), stop=(g == 2 * N_CORES - 1))
                    ys = work.tile([128, QCW], F32, tag="ys", bufs=4,
                                   name="ys")
                    nc.vector.tensor_copy(out=ys[:], in_=yp[:])
                    nc.scalar.dma_start(
                        yt.ap()[ec * 128:(ec + 1) * 128,
                                qc * QCW:(qc + 1) * QCW],
                        ys[:])

    nc.compile()
    return nc


def _prep_inputs(x, q_w, k_w, v_w, o_w, gate_w, gate_b):
    x = np.asarray(x, dtype=np.float32)
    xt = np.ascontiguousarray(x.reshape(S, D).T).astype(BF16)
    gwt = np.ascontiguousarray(np.asarray(gate_w, np.float32).T).astype(BF16)
    gb = np.asarray(gate_b, np.float32).reshape(HD, 1).copy()
    trim = np.triu(np.ones((128, 128), np.float32)).astype(BF16)
    in_maps = []
    for c in range(N_CORES):
        sl = slice(c * E, (c + 1) * E)
        in_maps.append({
            "xt": xt,
            "wqt": np.ascontiguousarray(np.asarray(q_w, np.float32)[sl, :].T).astype(BF16),
            "wkt": np.ascontiguousarray(np.asarray(k_w, np.float32)[sl, :].T).astype(BF16),
            "wvt": np.ascontiguousarray(np.asarray(v_w, np.float32)[sl, :].T).astype(BF16),
            "owt": np.ascontiguousarray(np.asarray(o_w, np.float32)[sl, :].T).astype(BF16),
            "gwt": gwt,
            "gb": gb,
            "trim": trim,
        })
    return in_maps


def _run(in_maps, **kwargs):
    if "nc" not in _CACHED:
        _CACHED["nc"] = _build()
    return run_bass_kernel_spmd(_CACHED["nc"], in_maps,
                                core_ids=list(range(N_CORES)), **kwargs)


def kernel(x, q_w, k_w, v_w, o_w, gate_w, gate_b):
    res = _run(_prep_inputs(x, q_w, k_w, v_w, o_w, gate_w, gate_b))
    yts = [res.results[c]["yt"] for c in range(N_CORES)]
    y_t = np.concatenate(yts, axis=0)          # [D(e), S]
    return np.ascontiguousarray(y_t.T, dtype=np.float32).reshape(1, S, D)


# revision 16
# speedup vs baseline: 1.0597x; 1.0597x over previous
"""GatedAttention Trainium2 kernel, 8-way tensor-parallel over heads.

Reference computation (B=1, S=2048, D=2048, H=16 heads, Hd=128):
  q,k,v = x @ {q,k,v}_w.T  (per-head split)
  scores = (q @ k.T) / sqrt(Hd), causal mask, softmax
  av = attn @ v
  gate = sigmoid(q @ gate_w.T + gate_b)       (per-head)
  y = concat_heads(av * gate) @ o_w.T

Sharding: 2 heads per core (column-parallel QKV/gate). The gated per-head
outputs are AllGathered in bf16 [feature, seq] layout, one AllGather per
512-wide q-chunk (both heads stacked: 256 rows x 512 cols, 256KB/rank).
o_proj is column-parallel over the gathered features; the host concatenates
the 8 output column slices.

Schedule notes:
 - Fully chunk-pipelined along the sequence: x^T is loaded one 512-column
   chunk at a time, and each chunk runs Q/K proj -> gates -> V proj ->
   attention -> store -> AllGather before the next chunk's attention. The
   first collective therefore triggers ~60us into the kernel and the four
   serialized collectives (~26us each on this LNC1 8-rank config) hide
   almost entirely under the remaining compute.
 - Softmax runs without max-subtraction (scores are small by construction).
   exp row-sums are accumulated per-block on the Vector engine (running
   tensor_add over the [j,q]-layout exp tiles) and reduced across the j
   partition axis with one gpsimd partition_all_reduce per block, keeping
   the PE free of M=1 ones-matmuls.
 - attds stores ride the gpsimd queue (the AllGather trigger that depends
   on them is right behind); the only DMAs on the sync queue after the
   input loads are the gathered-output reads, so a read blocked on its
   AllGather can never head-of-line-block an attds store.
"""

import numpy as np
import ml_dtypes

import concourse.bass as bass
import concourse.bass_isa as bass_isa
import concourse.mybir as mybir
import concourse.tile as tile
from concourse import bacc
from concourse.bass_utils import run_bass_kernel_spmd

BF16 = ml_dtypes.bfloat16
F32 = mybir.dt.float32
BF = mybir.dt.bfloat16
AF = mybir.ActivationFunctionType

N_CORES = 8
S = 2048          # sequence length
D = 2048          # model dim
H = 16            # total heads
HD = 128          # head dim
HPC = H // N_CORES                   # heads per core: 2
E = HPC * HD                         # 256 output dims per core
DC = D // 128                        # 16 contraction chunks
QCW = 512                            # q-chunk width
NQC = S // QCW                       # 4 q-chunks
SCALE = 1.0 / float(np.sqrt(HD))

_CACHED = {}


def _build(collective=True):
    nc = bacc.Bacc("TRN2", target_bir_lowering=False, debug=False,
                   num_devices=N_CORES if collective else 1,
                   enable_asserts=False)

    xt = nc.dram_tensor("xt", [D, S], BF, kind="ExternalInput")        # x^T
    wqt = nc.dram_tensor("wqt", [D, E], BF, kind="ExternalInput")      # q_w shard^T
    wkt = nc.dram_tensor("wkt", [D, E], BF, kind="ExternalInput")
    wvt = nc.dram_tensor("wvt", [D, E], BF, kind="ExternalInput")
    owt = nc.dram_tensor("owt", [D, E], BF, kind="ExternalInput")      # o_w shard^T
    gwt = nc.dram_tensor("gwt", [HD, HD], BF, kind="ExternalInput")    # gate_w^T
    gb = nc.dram_tensor("gb", [HD, 1], F32, kind="ExternalInput")      # gate bias
    trim = nc.dram_tensor("trim", [128, 128], BF, kind="ExternalInput")
    yt = nc.dram_tensor("yt", [E, S], F32, kind="ExternalOutput")      # y^T slice

    shared = "Shared" if collective else "Local"

    with tile.TileContext(nc) as tc:
        with tc.tile_pool(name="const", bufs=1) as const, \
             tc.tile_pool(name="work", bufs=2) as work, \
             tc.tile_pool(name="psum", bufs=1, space="PSUM") as psum, \
             tc.tile_pool(name="dram", bufs=1, space="DRAM") as dram:

            def pp(name):
                return psum.tile([128, QCW], F32, tag="pp", bufs=6, name=name)

            def pp_av(name):
                # attention-output accumulators get their own two banks:
                # they drain late (deferred epilogue reads them one block
                # later), and sharing the rotation would stall projection
                # chains on the epilogue's att-multiply
                return psum.tile([128, QCW], F32, tag="avp", bufs=2,
                                 name=name)

            # ---- weight loads ----
            wqts = const.tile([128, DC, E], BF, tag="wqts", name="wqts")
            wkts = const.tile([128, DC, E], BF, tag="wkts", name="wkts")
            wvts = const.tile([128, DC, E], BF, tag="wvts", name="wvts")
            owts = const.tile([128, DC, E], BF, tag="owts", name="owts")

            def _ldw(dst, src, dc):
                # one 64KB DMA per dc chunk so weight loads spread across
                # queues (a whole-tensor dma_start serializes ~1MB on one
                # queue at ~22GB/s)
                nc.sync.dma_start(
                    dst[:, dc, :],
                    src.ap()[dc * 128:(dc + 1) * 128, :])

            # x^T resident in SBUF, loaded in per-(q-chunk, dc) column tiles
            # (128KB each) so the transfers spread across all DMA queues in
            # q-chunk priority order and no projection chain ever catches
            # the stream mid-chunk
            xts = const.tile([128, DC, S], BF, tag="xts", name="xts")

            def emit_xload(qc):
                q0 = qc * QCW
                for d0 in range(DC):
                    nc.sync.dma_start(
                        xts[:, d0, q0:q0 + QCW],
                        xt.ap()[d0 * 128:(d0 + 1) * 128, q0:q0 + QCW])

            # front loads interleaved per-dc so the first Q/K chains are fed
            # with minimum latency: (wq[dc], x0[dc], wk[dc]) round-robin
            for dc in range(DC):
                _ldw(wqts, wqt, dc)
                nc.sync.dma_start(
                    xts[:, dc, 0:QCW], xt.ap()[dc * 128:(dc + 1) * 128, 0:QCW])
                _ldw(wkts, wkt, dc)

            gwts = const.tile([HD, HD], BF, tag="gwts", name="gwts")
            gbs = const.tile([HD, 1], F32, tag="gbs", name="gbs")
            tris = const.tile([128, 128], BF, tag="tris", name="tris")
            nc.sync.dma_start(gwts[:], gwt.ap())
            nc.sync.dma_start(gbs[:], gb.ap())
            nc.sync.dma_start(tris[:], trim.ap())
            for dc in range(DC):
                _ldw(wvts, wvt, dc)
            emit_xload(1)
            for dc in range(DC):
                _ldw(owts, owt, dc)
            emit_xload(2)
            emit_xload(3)

            # ---- persistent activation tiles ----
            qts = const.tile([128, HPC, S], BF, tag="qts", name="qts")
            kts = const.tile([128, HPC, S], BF, tag="kts", name="kts")
            gts = const.tile([128, HPC, S], BF, tag="gts", name="gts")
            vts = const.tile([128, DC, E], BF, tag="vts", name="vts")

            attds = [dram.tile([E, QCW], BF, tag=f"attd{qc}", name=f"attd{qc}")
                     for qc in range(NQC)]
            outds = [dram.tile([N_CORES * E, QCW], BF, tag=f"outd{qc}",
                               addr_space=shared, name=f"outd{qc}")
                     for qc in range(NQC)]

            def emit_proj(wts, outts, h, qc):
                # one 16-matmul chain: outts[:, h, qc] = W_h^T x[:, qc]
                q0 = qc * QCW
                ppt = pp("qp")
                for dc in range(DC):
                    nc.tensor.matmul(
                        ppt[:], wts[:, dc, h * 128:(h + 1) * 128],
                        xts[:, dc, q0:q0 + QCW], start=(dc == 0),
                        stop=(dc == DC - 1))
                nc.vector.tensor_copy(out=outts[:, h, q0:q0 + QCW], in_=ppt[:])

            def emit_vchain(sc16):
                # V: [s(128), e] natural layout, one 16-matmul chain
                vp = pp("vp")
                for dc in range(DC):
                    nc.tensor.matmul(
                        vp[:, :E],
                        xts[:, dc, sc16 * 128:(sc16 + 1) * 128],
                        wvts[:, dc, :], start=(dc == 0), stop=(dc == DC - 1))
                nc.vector.tensor_copy(out=vts[:, sc16, :], in_=vp[:, :E])

            def emit_gate(h, qc):
                q0 = qc * QCW
                gp = pp("gp")
                nc.tensor.matmul(gp[:], gwts[:], qts[:, h, q0:q0 + QCW],
                                 start=True, stop=True)
                nc.scalar.activation(gts[:, h, q0:q0 + QCW], gp[:],
                                     AF.Sigmoid, bias=gbs[:, 0:1])

            def emit_ag(qc):
                if collective:
                    nc.gpsimd.collective_compute(
                        "AllGather", mybir.AluOpType.bypass,
                        replica_groups=[list(range(N_CORES))],
                        ins=[attds[qc][:].opt()], outs=[outds[qc][:].opt()])
                else:
                    nc.sync.dma_start(outds[qc][0:E, :], attds[qc][:])

            # Software-pipelined across blocks: each block's last AV matmuls
            # and its epilogue are emitted after the NEXT chunk of PE work
            # (projection chain or next block's first scores/exp), so the PE
            # never idles waiting for the tail exp on ACT.
            pend = None   # deferred tail of the previous block

            def emit_tail_av(t, k):
                # deferred AV for jj_l-1 (k=0) or jj_l (k=1, stop)
                (h, qc, avp, acc, exts_l, s0s, jj_l) = t
                jj = jj_l - 1 + k
                s0 = s0s[k]
                nc.tensor.matmul(
                    avp[:, s0:], vts[:, jj, h * 128:(h + 1) * 128],
                    exts_l[jj % 3][:, s0:], start=False, stop=(k == 1))

            def emit_tail(t):
                (h, qc, avp, acc, exts_l, s0s, jj_l) = t
                q0 = qc * QCW
                # softmax denominator: reduce the running exp-sum across the
                # j partition axis on the (otherwise idle) Pool engine
                sumb = work.tile([128, QCW], F32, tag="sumb", bufs=2,
                                 name="sumb")
                nc.gpsimd.partition_all_reduce(sumb[:], acc[:], channels=128,
                                               reduce_op=bass_isa.ReduceOp.add)
                rb = work.tile([128, QCW], F32, tag="rb", bufs=2, name="rb")
                nc.vector.reciprocal_approx_fast(out=rb[:], in_=sumb[:])
                gn = work.tile([128, QCW], BF, tag="gn", bufs=2, name="gn")
                nc.vector.tensor_mul(gn[:], gts[:, h, q0:q0 + QCW], rb[:])
                att = work.tile([128, QCW], BF, tag="att", bufs=2, name="att")
                nc.vector.tensor_mul(att[:], avp[:], gn[:])
                nc.gpsimd.dma_start(attds[qc][h * HD:(h + 1) * HD, :], att[:])
                if h == HPC - 1:
                    emit_ag(qc)

            for qc in range(NQC):
                # the previous q-chunk's deferred tail rides after the K
                # chains: by then the ACT engine has caught up on the tail
                # exps, so the tail AVs never block the PE queue
                emit_proj(wqts, qts, 0, qc)
                emit_proj(wqts, qts, 1, qc)
                emit_proj(wkts, kts, 0, qc)
                if pend is not None:
                    emit_tail_av(pend, 0)
                emit_proj(wkts, kts, 1, qc)
                if pend is not None:
                    emit_tail_av(pend, 1)
                    emit_tail(pend)
                    pend = None
                emit_gate(0, qc)
                emit_gate(1, qc)
                for sc16 in range(4 * qc, 4 * qc + 4):
                    emit_vchain(sc16)

                for h in range(HPC):
                    q0 = qc * QCW
                    scps = [pp("scp") for _ in range(3)]
                    avp = pp_av("avp")
                    njj = 4 * qc + 4
                    exts = [work.tile([128, QCW], BF, tag="ext", bufs=6,
                                      name="ext") for _ in range(3)]
                    acc = work.tile([128, QCW], F32, tag="acc", bufs=2,
                                    name="acc")

                    def s0_of(jj):
                        return max(0, (jj - 4 * qc) * 128)

                    def emit_av(jj):
                        s0 = s0_of(jj)
                        nc.tensor.matmul(
                            avp[:, s0:], vts[:, jj, h * 128:(h + 1) * 128],
                            exts[jj % 3][:, s0:],
                            start=(jj == 0), stop=False)

                    # scores run two jj ahead of AV so the PE never waits on
                    # the exp->mask chain; the exp-sum accumulates on Vector
                    for jj in range(njj):
                        off = jj - 4 * qc
                        s0 = s0_of(jj)
                        scp = scps[jj % 3]
                        ext = exts[jj % 3]
                        nc.tensor.matmul(
                            scp[:, s0:], kts[:, h, jj * 128:(jj + 1) * 128],
                            qts[:, h, q0 + s0:q0 + QCW], start=True, stop=True)
                        nc.scalar.activation(ext[:, s0:], scp[:, s0:],
                                             AF.Exp, scale=SCALE)
                        if off >= 0:
                            nc.vector.tensor_mul(ext[:, s0:s0 + 128],
                                                 ext[:, s0:s0 + 128], tris[:])
                        if jj == 0:
                            nc.vector.tensor_copy(out=acc[:], in_=ext[:])
                        else:
                            nc.vector.tensor_add(acc[:, s0:], acc[:, s0:],
                                                 ext[:, s0:])
                        if pend is not None:
                            if jj == 0:
                                emit_tail_av(pend, 0)
                            elif jj == 1:
                                emit_tail_av(pend, 1)
                                emit_tail(pend)
                                pend = None
                        if jj >= 2:
                            emit_av(jj - 2)
                    pend = (h, qc, avp, acc, exts,
                            (s0_of(njj - 2), s0_of(njj - 1)), njj - 1)
            # flush the final block so the last store - and with it the
            # last AllGather - issues immediately
            emit_tail_av(pend, 0)
            emit_tail_av(pend, 1)
            emit_tail(pend)
            pend = None

            # ---- o_proj: y^T[e', s] = sum_f o_w[cs+e', f] out^T[f, s] ----
            # per q-chunk as the gathers land; the gathered reads are the
            # only DMAs left on the sync queue so their AllGather waits
            # cannot block anything else
            for qc in range(NQC):
                outsb = work.tile([128, 2 * N_CORES, QCW], BF, tag="outsb",
                                  bufs=2, name="outsb")
                # one DMA per 128-row feature chunk so the 2MB gathered read
                # spreads across all DMA queues instead of serializing on one
                for g in range(2 * N_CORES):
                    nc.sync.dma_start(
                        outsb[:, g, :],
                        outds[qc][g * 128:(g + 1) * 128, :])
                for ec in range(HPC):
                    yp = pp("yp")
                    for g in range(2 * N_CORES):
                        nc.tensor.matmul(
                            yp[:],
                            owts[:, g, ec * 128:(ec + 1) * 128],
                            outsb[:, g, :],
                            start=(g == 0), stop=(g == 2 * N_CORES - 1))
                    ys = work.tile([128, QCW], F32, tag="ys", bufs=4,
                                   name="ys")
                    nc.vector.tensor_copy(out=ys[:], in_=yp[:])
                    nc.scalar.dma_start(
                        yt.ap()[ec * 128:(ec + 1) * 128,
                                qc * QCW:(qc + 1) * QCW],
                        ys[:])

    nc.compile()
    return nc


def _prep_inputs(x, q_w, k_w, v_w, o_w, gate_w, gate_b):
    x = np.asarray(x, dtype=np.float32)
    xt = np.ascontiguousarray(x.reshape(S, D).T).astype(BF16)
    gwt = np.ascontiguousarray(np.asarray(gate_w, np.float32).T).astype(BF16)
    gb = np.asarray(gate_b, np.float32).reshape(HD, 1).copy()
    trim = np.triu(np.ones((128, 128), np.float32)).astype(BF16)
    in_maps = []
    for c in range(N_CORES):
        sl = slice(c * E, (c + 1) * E)
        in_maps.append({
            "xt": xt,
            "wqt": np.ascontiguousarray(np.asarray(q_w, np.float32)[sl, :].T).astype(BF16),
            "wkt": np.ascontiguousarray(np.asarray(k_w, np.float32)[sl, :].T).astype(BF16),
            "wvt": np.ascontiguousarray(np.asarray(v_w, np.float32)[sl, :].T).astype(BF16),
            "owt": np.ascontiguousarray(np.asarray(o_w, np.float32)[sl, :].T).astype(BF16),
            "gwt": gwt,
            "gb": gb,
            "trim": trim,
        })
    return in_maps


def _run(in_maps, **kwargs):
    if "nc" not in _CACHED:
        _CACHED["nc"] = _build()
    return run_bass_kernel_spmd(_CACHED["nc"], in_maps,
                                core_ids=list(range(N_CORES)), **kwargs)


def kernel(x, q_w, k_w, v_w, o_w, gate_w, gate_b):
    res = _run(_prep_inputs(x, q_w, k_w, v_w, o_w, gate_w, gate_b))
    yts = [res.results[c]["yt"] for c in range(N_CORES)]
    y_t = np.concatenate(yts, axis=0)          # [D(e), S]
    return np.ascontiguousarray(y_t.T, dtype=np.float32).reshape(1, S, D)


# revision 17
# speedup vs baseline: 1.0612x; 1.0014x over previous
"""GatedAttention Trainium2 kernel, 8-way tensor-parallel over heads.

Reference computation (B=1, S=2048, D=2048, H=16 heads, Hd=128):
  q,k,v = x @ {q,k,v}_w.T  (per-head split)
  scores = (q @ k.T) / sqrt(Hd), causal mask, softmax
  av = attn @ v
  gate = sigmoid(q @ gate_w.T + gate_b)       (per-head)
  y = concat_heads(av * gate) @ o_w.T

Sharding: 2 heads per core (column-parallel QKV/gate). The gated per-head
outputs are AllGathered in bf16 [feature, seq] layout, one AllGather per
q-chunk (both heads stacked). o_proj is column-parallel over the gathered
features; the host concatenates the 8 output column slices.

Schedule notes:
 - Fully chunk-pipelined along the sequence: x^T is loaded per q-chunk,
   and each chunk runs Q/K proj -> gates -> V proj -> attention -> store ->
   AllGather before the next chunk's attention. The first collective
   triggers ~80us in and the serialized collectives (~16us per 256KB rank
   share on this LNC1 8-rank config) hide under the remaining compute.
 - Chunk widths are [512,512,512,256,256]: the tail is collective-bound,
   so the last gather is halved (128KB/rank) and the second-to-last one
   fits in the idle window the quadratic attention cost otherwise leaves
   in the collective chain.
 - All HBM loads are issued as <=128KB per-dc DMAs: a whole-tensor
   dma_start serializes on ONE of the 16 DMA queues at ~22GB/s, which both
   delays the consumer and collides with collective windows.
 - Softmax runs without max-subtraction (scores are small by construction).
   exp row-sums are accumulated per-block on the Vector engine (running
   tensor_add over the [j,q]-layout exp tiles) and reduced across the j
   partition axis with one gpsimd partition_all_reduce per block, keeping
   the PE free of M=1 ones-matmuls.
 - The deferred tail AVs of each block ride after the NEXT chunk's K
   chains, giving the ACT engine time to drain its exp backlog so the tail
   never blocks the PE queue; attds stores ride the gpsimd queue (the
   AllGather trigger that depends on them is right behind); the gathered
   reads are the only DMAs on the sync queue after the input loads.
"""

import numpy as np
import ml_dtypes

import concourse.bass as bass
import concourse.bass_isa as bass_isa
import concourse.mybir as mybir
import concourse.tile as tile
from concourse import bacc
from concourse.bass_utils import run_bass_kernel_spmd

BF16 = ml_dtypes.bfloat16
F32 = mybir.dt.float32
BF = mybir.dt.bfloat16
AF = mybir.ActivationFunctionType

N_CORES = 8
S = 2048          # sequence length
D = 2048          # model dim
H = 16            # total heads
HD = 128          # head dim
HPC = H // N_CORES                   # heads per core: 2
E = HPC * HD                         # 256 output dims per core
DC = D // 128                        # 16 contraction chunks
SCALE = 1.0 / float(np.sqrt(HD))

# q-chunks: (q0, width). Tail chunks are narrower so the last AllGathers
# are smaller (the kernel tail is collective-latency-bound).
CHUNKS = [(0, 512), (512, 512), (1024, 512), (1536, 256), (1792, 256)]

_CACHED = {}


def _build(collective=True):
    nc = bacc.Bacc("TRN2", target_bir_lowering=False, debug=False,
                   num_devices=N_CORES if collective else 1,
                   enable_asserts=False)

    xt = nc.dram_tensor("xt", [D, S], BF, kind="ExternalInput")        # x^T
    wqt = nc.dram_tensor("wqt", [D, E], BF, kind="ExternalInput")      # q_w shard^T
    wkt = nc.dram_tensor("wkt", [D, E], BF, kind="ExternalInput")
    wvt = nc.dram_tensor("wvt", [D, E], BF, kind="ExternalInput")
    owt = nc.dram_tensor("owt", [D, E], BF, kind="ExternalInput")      # o_w shard^T
    gwt = nc.dram_tensor("gwt", [HD, HD], BF, kind="ExternalInput")    # gate_w^T
    gb = nc.dram_tensor("gb", [HD, 1], F32, kind="ExternalInput")      # gate bias
    trim = nc.dram_tensor("trim", [128, 128], BF, kind="ExternalInput")
    yt = nc.dram_tensor("yt", [E, S], F32, kind="ExternalOutput")      # y^T slice

    shared = "Shared" if collective else "Local"

    with tile.TileContext(nc) as tc:
        with tc.tile_pool(name="const", bufs=1) as const, \
             tc.tile_pool(name="work", bufs=2) as work, \
             tc.tile_pool(name="psum", bufs=1, space="PSUM") as psum, \
             tc.tile_pool(name="dram", bufs=1, space="DRAM") as dram:

            def pp(name):
                return psum.tile([128, 512], F32, tag="pp", bufs=6, name=name)

            def pp_av(name):
                # attention-output accumulators get their own two banks:
                # they drain late (deferred epilogue reads them one block
                # later), and sharing the rotation would stall projection
                # chains on the epilogue's att-multiply
                return psum.tile([128, 512], F32, tag="avp", bufs=2,
                                 name=name)

            # ---- weight + x loads, all split into <=128KB per-dc DMAs ----
            wqts = const.tile([128, DC, E], BF, tag="wqts", name="wqts")
            wkts = const.tile([128, DC, E], BF, tag="wkts", name="wkts")
            wvts = const.tile([128, DC, E], BF, tag="wvts", name="wvts")
            owts = const.tile([128, DC, E], BF, tag="owts", name="owts")
            xts = const.tile([128, DC, S], BF, tag="xts", name="xts")

            def _ldw(dst, src, dc):
                nc.sync.dma_start(
                    dst[:, dc, :],
                    src.ap()[dc * 128:(dc + 1) * 128, :])

            def emit_xload(ci):
                q0, qw = CHUNKS[ci]
                for d0 in range(DC):
                    nc.sync.dma_start(
                        xts[:, d0, q0:q0 + qw],
                        xt.ap()[d0 * 128:(d0 + 1) * 128, q0:q0 + qw])

            # front loads interleaved per-dc so the first Q/K chains are fed
            # with minimum latency: (wq[dc], x0[dc], wk[dc]) round-robin
            q00, qw0 = CHUNKS[0]
            for dc in range(DC):
                _ldw(wqts, wqt, dc)
                nc.sync.dma_start(
                    xts[:, dc, q00:q00 + qw0],
                    xt.ap()[dc * 128:(dc + 1) * 128, q00:q00 + qw0])
                _ldw(wkts, wkt, dc)

            gwts = const.tile([HD, HD], BF, tag="gwts", name="gwts")
            gbs = const.tile([HD, 1], F32, tag="gbs", name="gbs")
            tris = const.tile([128, 128], BF, tag="tris", name="tris")
            nc.sync.dma_start(gwts[:], gwt.ap())
            nc.sync.dma_start(gbs[:], gb.ap())
            nc.sync.dma_start(tris[:], trim.ap())
            for dc in range(DC):
                _ldw(wvts, wvt, dc)
            emit_xload(1)
            for dc in range(DC):
                _ldw(owts, owt, dc)
            for ci in range(2, len(CHUNKS)):
                emit_xload(ci)

            # ---- persistent activation tiles ----
            qts = const.tile([128, HPC, S], BF, tag="qts", name="qts")
            kts = const.tile([128, HPC, S], BF, tag="kts", name="kts")
            gts = const.tile([128, HPC, S], BF, tag="gts", name="gts")
            vts = const.tile([128, DC, E], BF, tag="vts", name="vts")

            attds = [dram.tile([E, qw], BF, tag=f"attd{ci}",
                               name=f"attd{ci}")
                     for ci, (q0, qw) in enumerate(CHUNKS)]
            outds = [dram.tile([N_CORES * E, qw], BF, tag=f"outd{ci}",
                               addr_space=shared, name=f"outd{ci}")
                     for ci, (q0, qw) in enumerate(CHUNKS)]

            def emit_proj(wts, outts, h, q0, qw):
                # one 16-matmul chain: outts[:, h, q-chunk] = W_h^T x-chunk
                ppt = pp("qp")
                for dc in range(DC):
                    nc.tensor.matmul(
                        ppt[:, :qw], wts[:, dc, h * 128:(h + 1) * 128],
                        xts[:, dc, q0:q0 + qw], start=(dc == 0),
                        stop=(dc == DC - 1))
                nc.vector.tensor_copy(out=outts[:, h, q0:q0 + qw],
                                      in_=ppt[:, :qw])

            def emit_vchain(sc16):
                # V: [s(128), e] natural layout, one 16-matmul chain
                vp = pp("vp")
                for dc in range(DC):
                    nc.tensor.matmul(
                        vp[:, :E],
                        xts[:, dc, sc16 * 128:(sc16 + 1) * 128],
                        wvts[:, dc, :], start=(dc == 0), stop=(dc == DC - 1))
                nc.vector.tensor_copy(out=vts[:, sc16, :], in_=vp[:, :E])

            def emit_gate(h, q0, qw):
                gp = pp("gp")
                nc.tensor.matmul(gp[:, :qw], gwts[:],
                                 qts[:, h, q0:q0 + qw],
                                 start=True, stop=True)
                nc.scalar.activation(gts[:, h, q0:q0 + qw], gp[:, :qw],
                                     AF.Sigmoid, bias=gbs[:, 0:1])

            def emit_ag(ci):
                if collective:
                    nc.gpsimd.collective_compute(
                        "AllGather", mybir.AluOpType.bypass,
                        replica_groups=[list(range(N_CORES))],
                        ins=[attds[ci][:].opt()], outs=[outds[ci][:].opt()])
                else:
                    nc.sync.dma_start(outds[ci][0:E, :], attds[ci][:])

            # Software-pipelined across blocks: each block's last AV matmuls
            # and its epilogue are emitted after the NEXT chunk of PE work,
            # so the PE never idles waiting for the tail exp on ACT.
            pend = None   # deferred tail of the previous block

            def emit_tail_av(t, k):
                # deferred AV for jj_l-1 (k=0) or jj_l (k=1, stop)
                (h, ci, avp, acc, exts_l, s0s, jj_l) = t
                q0, qw = CHUNKS[ci]
                jj = jj_l - 1 + k
                s0 = s0s[k]
                nc.tensor.matmul(
                    avp[:, s0:qw], vts[:, jj, h * 128:(h + 1) * 128],
                    exts_l[jj % 3][:, s0:qw], start=False, stop=(k == 1))

            def emit_tail(t):
                (h, ci, avp, acc, exts_l, s0s, jj_l) = t
                q0, qw = CHUNKS[ci]
                # softmax denominator: reduce the running exp-sum across the
                # j partition axis on the (otherwise idle) Pool engine
                sumb = work.tile([128, 512], F32, tag="sumb", bufs=2,
                                 name="sumb")
                nc.gpsimd.partition_all_reduce(sumb[:, :qw], acc[:, :qw],
                                               channels=128,
                                               reduce_op=bass_isa.ReduceOp.add)
                rb = work.tile([128, 512], F32, tag="rb", bufs=2, name="rb")
                nc.vector.reciprocal_approx_fast(out=rb[:, :qw],
                                                 in_=sumb[:, :qw])
                gn = work.tile([128, 512], BF, tag="gn", bufs=2, name="gn")
                nc.vector.tensor_mul(gn[:, :qw], gts[:, h, q0:q0 + qw],
                                     rb[:, :qw])
                att = work.tile([128, 512], BF, tag="att", bufs=2, name="att")
                nc.vector.tensor_mul(att[:, :qw], avp[:, :qw], gn[:, :qw])
                nc.gpsimd.dma_start(attds[ci][h * HD:(h + 1) * HD, :],
                                    att[:, :qw])
                if h == HPC - 1:
                    emit_ag(ci)

            prev_jend = 0
            for ci, (q0, qw) in enumerate(CHUNKS):
                jend = (q0 + qw) // 128
                # the previous q-chunk's deferred tail rides after the K
                # chains: by then the ACT engine has caught up on the tail
                # exps, so the tail AVs never block the PE queue
                emit_proj(wqts, qts, 0, q0, qw)
                emit_proj(wqts, qts, 1, q0, qw)
                emit_proj(wkts, kts, 0, q0, qw)
                if pend is not None:
                    emit_tail_av(pend, 0)
                emit_proj(wkts, kts, 1, q0, qw)
                if pend is not None:
                    emit_tail_av(pend, 1)
                    emit_tail(pend)
                    pend = None
                emit_gate(0, q0, qw)
                emit_gate(1, q0, qw)
                for sc16 in range(prev_jend, jend):
                    emit_vchain(sc16)
                prev_jend = jend

                for h in range(HPC):
                    scps = [pp("scp") for _ in range(3)]
                    avp = pp_av("avp")
                    njj = jend
                    exts = [work.tile([128, 512], BF, tag="ext", bufs=6,
                                      name="ext") for _ in range(3)]
                    acc = work.tile([128, 512], F32, tag="acc", bufs=2,
                                    name="acc")

                    def s0_of(jj):
                        return max(0, jj * 128 - q0)

                    def emit_av(jj):
                        s0 = s0_of(jj)
                        nc.tensor.matmul(
                            avp[:, s0:qw], vts[:, jj, h * 128:(h + 1) * 128],
                            exts[jj % 3][:, s0:qw],
                            start=(jj == 0), stop=False)

                    # scores run two jj ahead of AV so the PE never waits on
                    # the exp->mask chain; the exp-sum accumulates on Vector
                    for jj in range(njj):
                        s0 = s0_of(jj)
                        diag = jj * 128 >= q0
                        scp = scps[jj % 3]
                        ext = exts[jj % 3]
                        nc.tensor.matmul(
                            scp[:, s0:qw], kts[:, h, jj * 128:(jj + 1) * 128],
                            qts[:, h, q0 + s0:q0 + qw], start=True, stop=True)
                        nc.scalar.activation(ext[:, s0:qw], scp[:, s0:qw],
                                             AF.Exp, scale=SCALE)
                        if diag:
                            nc.vector.tensor_mul(ext[:, s0:s0 + 128],
                                                 ext[:, s0:s0 + 128], tris[:])
                        if jj == 0:
                            nc.vector.tensor_copy(out=acc[:, :qw],
                                                  in_=ext[:, :qw])
                        else:
                            nc.vector.tensor_add(acc[:, s0:qw], acc[:, s0:qw],
                                                 ext[:, s0:qw])
                        if pend is not None:
                            if jj == 0:
                                emit_tail_av(pend, 0)
                            elif jj == 1:
                                emit_tail_av(pend, 1)
                                emit_tail(pend)
                                pend = None
                        if jj >= 2:
                            emit_av(jj - 2)
                    pend = (h, ci, avp, acc, exts,
                            (s0_of(njj - 2), s0_of(njj - 1)), njj - 1)
            # flush the final block so the last store - and with it the
            # last AllGather - issues immediately
            emit_tail_av(pend, 0)
            emit_tail_av(pend, 1)
            emit_tail(pend)
            pend = None

            # ---- o_proj: y^T[e', s] = sum_f o_w[cs+e', f] out^T[f, s] ----
            # per q-chunk as the gathers land; the gathered reads are the
            # only DMAs left on the sync queue so their AllGather waits
            # cannot block anything else
            for ci, (q0, qw) in enumerate(CHUNKS):
                outsb = work.tile([128, 2 * N_CORES, 512], BF, tag="outsb",
                                  bufs=2, name="outsb")
                # one DMA per 128-row feature chunk so the gathered read
                # spreads across the DMA queues instead of serializing
                for g in range(2 * N_CORES):
                    nc.sync.dma_start(
                        outsb[:, g, :qw],
                        outds[ci][g * 128:(g + 1) * 128, :])
                for ec in range(HPC):
                    yp = pp("yp")
                    for g in range(2 * N_CORES):
                        nc.tensor.matmul(
                            yp[:, :qw],
                            owts[:, g, ec * 128:(ec + 1) * 128],
                            outsb[:, g, :qw],
                            start=(g == 0), stop=(g == 2 * N_CORES - 1))
                    ys = work.tile([128, 512], F32, tag="ys", bufs=4,
                                   name="ys")
                    nc.vector.tensor_copy(out=ys[:, :qw], in_=yp[:, :qw])
                    # 128-col (64KB) store pieces: a single 256KB f32 store
                    # would serialize ~11us on one DMA queue at the very tail
                    for p0 in range(0, qw, 128):
                        nc.scalar.dma_start(
                            yt.ap()[ec * 128:(ec + 1) * 128,
                                    q0 + p0:q0 + p0 + 128],
                            ys[:, p0:p0 + 128])

    nc.compile()
    return nc


def _prep_inputs(x, q_w, k_w, v_w, o_w, gate_w, gate_b):
    x = np.asarray(x, dtype=np.float32)
    xt = np.ascontiguousarray(x.reshape(S, D).T).astype(BF16)
    gwt = np.ascontiguousarray(np.asarray(gate_w, np.float32).T).astype(BF16)
    gb = np.asarray(gate_b, np.float32).reshape(HD, 1).copy()
    trim = np.triu(np.ones((128, 128), np.float32)).astype(BF16)
    in_maps = []
    for c in range(N_CORES):
        sl = slice(c * E, (c + 1) * E)
        in_maps.append({
            "xt": xt,
            "wqt": np.ascontiguousarray(np.asarray(q_w, np.float32)[sl, :].T).astype(BF16),
            "wkt": np.ascontiguousarray(np.asarray(k_w, np.float32)[sl, :].T).astype(BF16),
            "wvt": np.ascontiguousarray(np.asarray(v_w, np.float32)[sl, :].T).astype(BF16),
            "owt": np.ascontiguousarray(np.asarray(o_w, np.float32)[sl, :].T).astype(BF16),
            "gwt": gwt,
            "gb": gb,
            "trim": trim,
        })
    return in_maps


def _run(in_maps, **kwargs):
    if "nc" not in _CACHED:
        _CACHED["nc"] = _build()
    return run_bass_kernel_spmd(_CACHED["nc"], in_maps,
                                core_ids=list(range(N_CORES)), **kwargs)


def kernel(x, q_w, k_w, v_w, o_w, gate_w, gate_b):
    res = _run(_prep_inputs(x, q_w, k_w, v_w, o_w, gate_w, gate_b))
    yts = [res.results[c]["yt"] for c in range(N_CORES)]
    y_t = np.concatenate(yts, axis=0)          # [D(e), S]
    return np.ascontiguousarray(y_t.T, dtype=np.float32).reshape(1, S, D)
